# revision 33
# baseline (speedup 1.0000x reference)
"""Trainium2 Bass kernel for nn_Attention_28930899706081 (sparse_attention).

Reference computation:
  k1 = l2norm_c(Wqk @ fmap1), k2 = l2norm_c(Wqk @ fmap2), q = l2norm_c(Wqk @ dmap)
  sim_i = q^T k_i per batch  -> [b, n, n] with n = h*w = 4096
  attn_i = softmax(sim_i, axis=-1)[:, None]  -> [b, 1, n, n]
  returns (attn1, attn2)

Sharding: 8 cores; core i handles batch b = i//4, K-map m = (i%4)//2 and
query-row half h = i%2 (2048 of 4096 rows). Each core computes the full
normalized K for its (batch, map) and its row half of that map's sim+softmax.

Engine budget: ACT (ScalarE) is the bottleneck (exp is ACT-only, 1
elem/cycle/lane), so ACT does only Abs_reciprocal_sqrt (phase A) and Exp
(+accumulator row-sums, phase B). Square/copy run on DVE, the K/Q normalize
muls on GpSimd. The last 512 columns of each row-tile's exp are offloaded to
DVE as (1+x/8)^8 in fp16 (|sim|<=1, typ |sim|~0.09, so the approx error is
~1e-3 rms); tensor_tensor_reduce folds those columns' row-sum on top of the
ACT accumulators for free. A dummy-matmul warmup burst at kernel start trips
the PE HAM clock gate to 2.4 GHz before real matmuls begin, keeping sim
production ahead of exp consumption. All Abs_reciprocal_sqrt strictly precede
all Exp so the ACT table set loads exactly twice. |sim| <= 1 so softmax needs
no max subtraction. Output is written bf16 and upcast on the host.
"""

import numpy as np
import ml_dtypes

B, C, H, W, D = 2, 256, 64, 64, 128
N = H * W  # 4096
QR = N // 2  # 2048 query rows per core
N_CORES = 8

_cached = {}


def _build():
    import concourse.mybir as mybir
    import concourse.tile as tile
    from concourse.tile_rust import add_dep_helper
    from concourse import bacc
    from contextlib import ExitStack

    f32 = mybir.dt.float32
    f16 = mybir.dt.float16
    bf16 = mybir.dt.bfloat16
    AF = mybir.ActivationFunctionType
    ALU = mybir.AluOpType

    nc = bacc.Bacc(
        "TRN2",
        target_bir_lowering=False,
        debug=False,
        enable_asserts=False,
        num_devices=N_CORES,
    )

    fm_ext = nc.dram_tensor("fm", [C, N], bf16, kind="ExternalInput").ap()
    xq_ext = nc.dram_tensor("xq", [C, QR], bf16, kind="ExternalInput").ap()
    wqkT_ext = nc.dram_tensor("wqkT", [C, D], bf16, kind="ExternalInput").ap()
    out_ext = nc.dram_tensor("out", [QR, N], bf16, kind="ExternalOutput").ap()

    PCH = 512  # f32 PSUM bank (matmul max free dim for f32 out)
    XCH = 1024  # phase-A column chunk
    CH = 2048  # phase-B sim chunk (4 PSUM banks)
    U = 384  # cols per row-tile whose exp runs on DVE instead of ACT
    WARM_MM = 18  # dummy matmuls to trip the PE HAM clock gate at start
    # (~7.7us of sustained PE activity = 2+ full HAM windows, so the clock
    # gate reliably opens regardless of the free-running window phase)

    with tile.TileContext(nc) as tc, ExitStack() as ctx:
        consts = ctx.enter_context(tc.tile_pool(name="consts", bufs=1))
        xin = ctx.enter_context(tc.tile_pool(name="xin", bufs=1))
        ya_pool = ctx.enter_context(tc.tile_pool(name="ya", bufs=3))
        rk_pool = ctx.enter_context(tc.tile_pool(name="rk", bufs=3))
        kn_pool = ctx.enter_context(tc.tile_pool(name="kn", bufs=1))
        e_pool = ctx.enter_context(tc.tile_pool(name="epool", bufs=2))
        t_pool = ctx.enter_context(tc.tile_pool(name="tpool", bufs=2))
        attn_pool = ctx.enter_context(tc.tile_pool(name="attn", bufs=3))
        stat_pool = ctx.enter_context(tc.tile_pool(name="stat", bufs=8))

        # constants + inputs (big contiguous DMAs)
        wqkT_sb = [
            consts.tile([128, D], bf16, tag=f"wqkT{k}", name=f"wqkT{k}")
            for k in range(2)
        ]
        nc.sync.dma_start(out=wqkT_sb[0][:], in_=wqkT_ext[0:128, :])
        nc.sync.dma_start(out=wqkT_sb[1][:], in_=wqkT_ext[128:256, :])
        fm_sb = [
            xin.tile([128, N], bf16, tag=f"fm{k}", name=f"fm{k}") for k in range(2)
        ]
        # fm on the ACT HWDGE ring so it loads in parallel with the sync-ring
        # transfers (ACT is idle this early)
        nc.scalar.dma_start(out=fm_sb[0][:], in_=fm_ext[0:128, :])
        nc.scalar.dma_start(out=fm_sb[1][:], in_=fm_ext[128:256, :])
        xq_sb = [
            xin.tile([128, QR], bf16, tag=f"xq{k}", name=f"xq{k}") for k in range(2)
        ]
        nc.sync.dma_start(out=xq_sb[0][:], in_=xq_ext[0:128, :])
        nc.sync.dma_start(out=xq_sb[1][:], in_=xq_ext[128:256, :])

        ones_sb = consts.tile([128, 128], bf16, tag="ones", name="ones")
        nc.vector.memset(ones_sb[:], 1.0)
        warm_src = consts.tile([128, PCH], bf16, tag="wsrc", name="wsrc")
        nc.vector.memset(warm_src[:], 0.0)
        # prime the ACT table set (abs_reciprocal_sqrt_and_small) during the
        # initial DMA wait
        warm = consts.tile([128, 1], f32, tag="warm", name="warm")
        nc.scalar.activation(out=warm[:], in_=ones_sb[:, 0:1], func=AF.Abs_reciprocal_sqrt)

        # PE HAM warmup: ~8 cold matmuls (~3.4us) trip the clock gate to
        # 2.4 GHz; the rest run warm. All during the initial input-DMA wait.
        with tc.tile_pool(name="warm_psum", bufs=1, space="PSUM") as warm_psum:
            wps = warm_psum.tile([128, PCH], f32, tag="wps", name="wps")
            for _ in range(WARM_MM):
                nc.tensor.matmul(wps[:], ones_sb[:], warm_src[:], start=True, stop=True)

        last_rk = None

        with tc.tile_pool(name="proj_psum", bufs=2, space="PSUM") as proj_psum, \
             tc.tile_pool(name="n2_psum", bufs=1, space="PSUM") as n2_psum:

            def emit_chunk(x_lo, x_hi, xn, h0, rscale):
                """project + l2-normalize one 1024-col chunk; ACT does only
                the rsqrt.

                rscale: free immediate folded into the rsqrt (rk =
                1/sqrt(rscale*n2)). The q side uses rscale=16 so sim comes
                out of the PE as sim/4 — the DVE exp path then needs only a
                single tensor_scalar_add, and ACT exp uses scale=4.
                """
                nonlocal last_rk
                ps = proj_psum.tile([128, XCH], f32, tag="proj", name="pps")
                for c in range(XCH // PCH):
                    sl = slice(c * PCH, (c + 1) * PCH)
                    xsl = slice(h0 + c * PCH, h0 + (c + 1) * PCH)
                    nc.tensor.matmul(
                        ps[:, sl], wqkT_sb[0][:], x_lo[:, xsl], start=True, stop=False
                    )
                    nc.tensor.matmul(
                        ps[:, sl], wqkT_sb[1][:], x_hi[:, xsl], start=False, stop=True
                    )
                # PSUM evacuation on ACT (idle during phase A; Copy is in
                # every ACT table set so no table reload)
                y_bf = ya_pool.tile([128, XCH], bf16, tag="ybf", name="y_bf")
                nc.scalar.activation(out=y_bf[:], in_=ps[:], func=AF.Copy)
                ysq = ya_pool.tile([128, XCH], bf16, tag="ysq", name="ysq")
                nc.vector.tensor_mul(ysq[:], y_bf[:], y_bf[:])
                nps = n2_psum.tile([128, XCH], f32, tag="n2", name="nps")
                for c in range(XCH // PCH):
                    sl = slice(c * PCH, (c + 1) * PCH)
                    nc.tensor.matmul(
                        nps[:, sl], ones_sb[:], ysq[:, sl], start=True, stop=True
                    )
                # rk = (rscale*n2)^-0.5, broadcast across partitions
                rk = rk_pool.tile([128, XCH], bf16, tag="rk", name="rk")
                last_rk = nc.scalar.activation(
                    out=rk[:], in_=nps[:], func=AF.Abs_reciprocal_sqrt,
                    scale=rscale,
                )
                nc.vector.tensor_mul(xn[:, h0 : h0 + XCH], y_bf[:], rk[:])

            qn = kn_pool.tile([128, QR], bf16, tag="qn", name="qn")
            km = kn_pool.tile([128, N], bf16, tag="km", name="km")
            for h0 in range(0, N, XCH):
                emit_chunk(fm_sb[0], fm_sb[1], km, h0, 1.0)
            for h0 in range(0, QR, XCH):
                emit_chunk(xq_sb[0], xq_sb[1], qn, h0, 16.0)

        with tc.tile_pool(name="sim_psum", bufs=2, space="PSUM") as sim_psum:
            first_exp = None
            for t in range(QR // 128):
                lhsT = qn[:, t * 128 : (t + 1) * 128]
                e = e_pool.tile([128, N], bf16, tag="e", name="e")
                stile = stat_pool.tile([128, 3], f32, tag="stile", name="stile")
                ps_last = None
                for j in range(N // CH):
                    ps = sim_psum.tile([128, CH], f32, tag="sim", name="sim_ps")
                    for c in range(CH // PCH):
                        csl = slice(j * CH + c * PCH, j * CH + (c + 1) * PCH)
                        nc.tensor.matmul(
                            ps[:, c * PCH : (c + 1) * PCH],
                            lhsT,
                            km[:, csl],
                            start=True,
                            stop=True,
                        )
                    # ACT exp (last U cols of the tile go to DVE instead).
                    # PSUM holds sim/4 (q-norm folded scale), so exp scale=4.
                    ecols = CH if j == 0 else CH - U
                    ex = nc.scalar.activation(
                        out=e[:, j * CH : j * CH + ecols],
                        in_=ps[:, 0:ecols],
                        func=AF.Exp,
                        scale=4.0,
                        accum_out=stile[:, j : j + 1],
                    )
                    if first_exp is None:
                        first_exp = ex
                        # keep ACT table loads to 2: all Abs_reciprocal_sqrt
                        # strictly before any Exp
                        add_dep_helper(
                            ex.ins, last_rk.ins, sync=False,
                            reason="order rk (ars table) before exp table load",
                        )
                    ps_last = ps
                # DVE exp for the last U cols: e = (1 + x/4)^4 in bf16;
                # PSUM already holds x/4
                t1 = t_pool.tile([128, U], bf16, tag="t1", name="t1")
                nc.vector.tensor_scalar_add(t1[:], ps_last[:, CH - U : CH], 1.0)
                t2 = t_pool.tile([128, U], bf16, tag="t2", name="t2")
                nc.vector.tensor_mul(t2[:], t1[:], t1[:])
                nc.vector.tensor_mul(e[:, N - U : N], t2[:], t2[:])
                # the DVE columns' row-sum lands in stile[:,2]; one reduce
                # then covers both ACT accumulators and the DVE part
                nc.vector.reduce_sum(
                    stile[:, 2:3], e[:, N - U : N], axis=mybir.AxisListType.X
                )
                ztot = stat_pool.tile([128, 1], f32, tag="ztot", name="ztot")
                nc.vector.reduce_sum(ztot[:], stile[:], axis=mybir.AxisListType.X)
                recip = stat_pool.tile([128, 1], f32, tag="recip", name="recip")
                nc.vector.reciprocal(recip[:], ztot[:])
                attn = attn_pool.tile([128, N], bf16, tag="attn", name="attn")
                nc.vector.tensor_scalar_mul(attn[:], e[:], recip[:])
                nc.sync.dma_start(
                    out=out_ext[t * 128 : (t + 1) * 128, :],
                    in_=attn[:],
                )

    nc.compile()
    return nc


def _get_nc():
    if "nc" not in _cached:
        _cached["nc"] = _build()
    return _cached["nc"]


def _in_maps(fmap1, fmap2, dmap, Wqk):
    bf = ml_dtypes.bfloat16
    f1r = np.asarray(fmap1, dtype=np.float32).reshape(B, C, N)
    f2r = np.asarray(fmap2, dtype=np.float32).reshape(B, C, N)
    dqr = np.asarray(dmap, dtype=np.float32).reshape(B, C, N)
    wT = np.ascontiguousarray(np.asarray(Wqk, dtype=np.float32).T).astype(bf)
    fr = [f1r, f2r]

    in_maps = []
    for i in range(N_CORES):
        b = i // 4
        m = (i % 4) // 2
        h = i % 2
        in_maps.append(
            {
                "fm": np.ascontiguousarray(fr[m][b]).astype(bf),
                "xq": np.ascontiguousarray(
                    dqr[b][:, h * QR : (h + 1) * QR]
                ).astype(bf),
                "wqkT": wT,
            }
        )
    return in_maps


def kernel(fmap1, fmap2, dmap, Wqk):
    from concourse.bass_utils import run_bass_kernel_spmd

    in_maps = _in_maps(fmap1, fmap2, dmap, Wqk)
    nc = _get_nc()
    res = run_bass_kernel_spmd(nc, in_maps, core_ids=list(range(N_CORES)))
    _cached["last_result"] = res

    attns = [
        np.empty((B, 1, N, N), dtype=np.float32),
        np.empty((B, 1, N, N), dtype=np.float32),
    ]
    for i in range(N_CORES):
        b = i // 4
        m = (i % 4) // 2
        h = i % 2
        o = res.results[i]["out"]
        attns[m][b, 0, h * QR : (h + 1) * QR, :] = o.astype(np.float32)
    return (attns[0], attns[1])


# revision 39
# speedup vs baseline: 1.2014x; 1.2014x over previous
"""Trainium2 Bass kernel for nn_Attention_28930899706081 (sparse_attention).

Reference computation:
  k1 = l2norm_c(Wqk @ fmap1), k2 = l2norm_c(Wqk @ fmap2), q = l2norm_c(Wqk @ dmap)
  sim_i = q^T k_i per batch  -> [b, n, n] with n = h*w = 4096
  attn_i = softmax(sim_i, axis=-1)[:, None]  -> [b, 1, n, n]
  returns (attn1, attn2)

Sharding: 8 cores; core i handles batch b = i//4, K-map m = (i%4)//2 and
query-row half h = i%2 (2048 of 4096 rows). Each core computes the full
normalized K for its (batch, map) and its row half of that map's sim+softmax.

Engine budget: ACT (ScalarE) is the bottleneck (exp is ACT-only, 1
elem/cycle/lane), so ACT does only Abs_reciprocal_sqrt (phase A) and Exp
(+accumulator row-sums, phase B). Square/copy run on DVE, the K/Q normalize
muls on GpSimd. The last 512 columns of each row-tile's exp are offloaded to
DVE as (1+x/8)^8 in fp16 (|sim|<=1, typ |sim|~0.09, so the approx error is
~1e-3 rms); tensor_tensor_reduce folds those columns' row-sum on top of the
ACT accumulators for free. A dummy-matmul warmup burst at kernel start trips
the PE HAM clock gate to 2.4 GHz before real matmuls begin, keeping sim
production ahead of exp consumption. All Abs_reciprocal_sqrt strictly precede
all Exp so the ACT table set loads exactly twice. |sim| <= 1 so softmax needs
no max subtraction. Output is written bf16 and upcast on the host.
"""

import numpy as np
import ml_dtypes

B, C, H, W, D = 2, 256, 64, 64, 128
N = H * W  # 4096
QR = N // 2  # 2048 query rows per core
N_CORES = 8

_cached = {}


def _build():
    import concourse.mybir as mybir
    import concourse.tile as tile
    from concourse.tile_rust import add_dep_helper
    from concourse import bacc
    from contextlib import ExitStack

    f32 = mybir.dt.float32
    f16 = mybir.dt.float16
    bf16 = mybir.dt.bfloat16
    AF = mybir.ActivationFunctionType
    ALU = mybir.AluOpType

    nc = bacc.Bacc(
        "TRN2",
        target_bir_lowering=False,
        debug=False,
        enable_asserts=False,
        num_devices=N_CORES,
    )

    fm_ext = nc.dram_tensor("fm", [C, N], bf16, kind="ExternalInput").ap()
    xq_ext = nc.dram_tensor("xq", [C, QR], bf16, kind="ExternalInput").ap()
    wqkT_ext = nc.dram_tensor("wqkT", [C, D], bf16, kind="ExternalInput").ap()
    out_ext = nc.dram_tensor("out", [QR, N], bf16, kind="ExternalOutput").ap()

    PCH = 512  # f32 PSUM bank (matmul max free dim for f32 out)
    XCH = 1024  # phase-A column chunk
    CH = 2048  # phase-B sim chunk (4 PSUM banks)
    U = 384  # cols per row-tile whose exp runs on DVE instead of ACT
    WARM_MM = 12  # dummy matmuls to trip the PE HAM clock gate at start

    with tile.TileContext(nc) as tc, ExitStack() as ctx:
        consts = ctx.enter_context(tc.tile_pool(name="consts", bufs=1))
        xin = ctx.enter_context(tc.tile_pool(name="xin", bufs=1))
        ya_pool = ctx.enter_context(tc.tile_pool(name="ya", bufs=3))
        rk_pool = ctx.enter_context(tc.tile_pool(name="rk", bufs=3))
        kn_pool = ctx.enter_context(tc.tile_pool(name="kn", bufs=1))
        e_pool = ctx.enter_context(tc.tile_pool(name="epool", bufs=3))
        t_pool = ctx.enter_context(tc.tile_pool(name="tpool", bufs=2))
        attn_pool = ctx.enter_context(tc.tile_pool(name="attn", bufs=3))
        stat_pool = ctx.enter_context(tc.tile_pool(name="stat", bufs=8))

        # constants + inputs (big contiguous DMAs)
        wqkT_sb = [
            consts.tile([128, D], bf16, tag=f"wqkT{k}", name=f"wqkT{k}")
            for k in range(2)
        ]
        nc.sync.dma_start(out=wqkT_sb[0][:], in_=wqkT_ext[0:128, :])
        nc.sync.dma_start(out=wqkT_sb[1][:], in_=wqkT_ext[128:256, :])
        # Channel-half 0 of each input rides the sync HWDGE ring, half 1 the
        # ACT ring, and fm is chunked at 1024 cols: the first projection
        # chunk needs BOTH halves of its columns, and this way they arrive
        # in parallel (~8.6us) instead of serialized on one ring (~13us).
        fm_sb = [
            xin.tile([128, N], bf16, tag=f"fm{k}", name=f"fm{k}") for k in range(2)
        ]
        xq_sb = [
            xin.tile([128, QR], bf16, tag=f"xq{k}", name=f"xq{k}") for k in range(2)
        ]
        for cc in range(N // XCH):
            csl = slice(cc * XCH, (cc + 1) * XCH)
            nc.sync.dma_start(out=fm_sb[0][:, csl], in_=fm_ext[0:128, csl])
            nc.scalar.dma_start(out=fm_sb[1][:, csl], in_=fm_ext[128:256, csl])
        nc.sync.dma_start(out=xq_sb[0][:], in_=xq_ext[0:128, :])
        nc.scalar.dma_start(out=xq_sb[1][:], in_=xq_ext[128:256, :])

        ones_sb = consts.tile([128, 128], bf16, tag="ones", name="ones")
        nc.vector.memset(ones_sb[:], 1.0)
        warm_src = consts.tile([128, PCH], bf16, tag="wsrc", name="wsrc")
        nc.vector.memset(warm_src[:], 0.0)
        # prime the ACT table set (abs_reciprocal_sqrt_and_small) during the
        # initial DMA wait
        warm = consts.tile([128, 1], f32, tag="warm", name="warm")
        nc.scalar.activation(out=warm[:], in_=ones_sb[:, 0:1], func=AF.Abs_reciprocal_sqrt)

        # PE HAM warmup: ~8 cold matmuls (~3.4us) trip the clock gate to
        # 2.4 GHz; the rest run warm. All during the initial input-DMA wait.
        with tc.tile_pool(name="warm_psum", bufs=1, space="PSUM") as warm_psum:
            wps = warm_psum.tile([128, PCH], f32, tag="wps", name="wps")
            for _ in range(WARM_MM):
                nc.tensor.matmul(wps[:], ones_sb[:], warm_src[:], start=True, stop=True)

        last_rk = None

        with tc.tile_pool(name="proj_psum", bufs=2, space="PSUM") as proj_psum, \
             tc.tile_pool(name="n2_psum", bufs=1, space="PSUM") as n2_psum:

            def emit_chunk(x_lo, x_hi, xn, h0, rscale, mul_eng):
                """project + l2-normalize one 1024-col chunk; ACT does only
                the rsqrt.

                rscale: free immediate folded into the rsqrt (rk =
                1/sqrt(rscale*n2)). The q side uses rscale=16 so sim comes
                out of the PE as sim/4 — the DVE exp path then needs only a
                single tensor_scalar_add, and ACT exp uses scale=4.
                """
                nonlocal last_rk
                ps = proj_psum.tile([128, XCH], f32, tag="proj", name="pps")
                for c in range(XCH // PCH):
                    sl = slice(c * PCH, (c + 1) * PCH)
                    xsl = slice(h0 + c * PCH, h0 + (c + 1) * PCH)
                    nc.tensor.matmul(
                        ps[:, sl], wqkT_sb[0][:], x_lo[:, xsl], start=True, stop=False
                    )
                    nc.tensor.matmul(
                        ps[:, sl], wqkT_sb[1][:], x_hi[:, xsl], start=False, stop=True
                    )
                # PSUM evacuation on DVE: the pre-exp critical path is
                # proj -> copy -> square -> norm-matmul -> rsqrt, and DVE at
                # ~1.9us/chunk beats routing the copy through ACT (~2.5)
                y_bf = ya_pool.tile([128, XCH], bf16, tag="ybf", name="y_bf")
                nc.vector.tensor_copy(y_bf[:], ps[:])
                ysq = ya_pool.tile([128, XCH], bf16, tag="ysq", name="ysq")
                nc.vector.tensor_mul(ysq[:], y_bf[:], y_bf[:])
                nps = n2_psum.tile([128, XCH], f32, tag="n2", name="nps")
                for c in range(XCH // PCH):
                    sl = slice(c * PCH, (c + 1) * PCH)
                    nc.tensor.matmul(
                        nps[:, sl], ones_sb[:], ysq[:, sl], start=True, stop=True
                    )
                # rk = (rscale*n2)^-0.5, broadcast across partitions
                rk = rk_pool.tile([128, XCH], bf16, tag="rk", name="rk")
                last_rk = nc.scalar.activation(
                    out=rk[:], in_=nps[:], func=AF.Abs_reciprocal_sqrt,
                    scale=rscale,
                )
                # the normalize mul is OFF the pre-exp path (phase B needs it,
                # the rsqrt chain doesn't): km's go to the idle GpSimd, qn's
                # stay on DVE (gpsimd processes the four km muls serially and
                # would otherwise gate tile 0)
                mul_eng(xn[:, h0 : h0 + XCH], y_bf[:], rk[:])

            qn = kn_pool.tile([128, QR], bf16, tag="qn", name="qn")
            km = kn_pool.tile([128, N], bf16, tag="km", name="km")
            for h0 in range(0, N, XCH):
                emit_chunk(fm_sb[0], fm_sb[1], km, h0, 1.0, nc.gpsimd.tensor_mul)
            for h0 in range(0, QR, XCH):
                emit_chunk(xq_sb[0], xq_sb[1], qn, h0, 16.0, nc.vector.tensor_mul)

        with tc.tile_pool(name="sim_psum", bufs=2, space="PSUM") as sim_psum:
            first_exp = None
            for t in range(QR // 128):
                lhsT = qn[:, t * 128 : (t + 1) * 128]
                e = e_pool.tile([128, N], bf16, tag="e", name="e")
                stile = stat_pool.tile([128, 3], f32, tag="stile", name="stile")
                ps_last = None
                for j in range(N // CH):
                    ps = sim_psum.tile([128, CH], f32, tag="sim", name="sim_ps")
                    for c in range(CH // PCH):
                        csl = slice(j * CH + c * PCH, j * CH + (c + 1) * PCH)
                        nc.tensor.matmul(
                            ps[:, c * PCH : (c + 1) * PCH],
                            lhsT,
                            km[:, csl],
                            start=True,
                            stop=True,
                        )
                    # ACT exp (last U cols of the tile go to DVE instead).
                    # PSUM holds sim/4 (q-norm folded scale), so exp scale=4.
                    ecols = CH if j == 0 else CH - U
                    ex = nc.scalar.activation(
                        out=e[:, j * CH : j * CH + ecols],
                        in_=ps[:, 0:ecols],
                        func=AF.Exp,
                        scale=4.0,
                        accum_out=stile[:, j : j + 1],
                    )
                    if first_exp is None:
                        first_exp = ex
                        # keep ACT table loads to 2: all Abs_reciprocal_sqrt
                        # strictly before any Exp
                        add_dep_helper(
                            ex.ins, last_rk.ins, sync=False,
                            reason="order rk (ars table) before exp table load",
                        )
                    ps_last = ps
                # DVE exp for the last U cols: e = (1 + x/4)^4 in bf16;
                # PSUM already holds x/4
                t1 = t_pool.tile([128, U], bf16, tag="t1", name="t1")
                nc.vector.tensor_scalar_add(t1[:], ps_last[:, CH - U : CH], 1.0)
                t2 = t_pool.tile([128, U], bf16, tag="t2", name="t2")
                nc.vector.tensor_mul(t2[:], t1[:], t1[:])
                nc.vector.tensor_mul(e[:, N - U : N], t2[:], t2[:])
                # the DVE columns' row-sum lands in stile[:,2]; one reduce
                # then covers both ACT accumulators and the DVE part
                nc.vector.reduce_sum(
                    stile[:, 2:3], e[:, N - U : N], axis=mybir.AxisListType.X
                )
                ztot = stat_pool.tile([128, 1], f32, tag="ztot", name="ztot")
                nc.vector.reduce_sum(ztot[:], stile[:], axis=mybir.AxisListType.X)
                recip = stat_pool.tile([128, 1], f32, tag="recip", name="recip")
                nc.vector.reciprocal(recip[:], ztot[:])
                attn = attn_pool.tile([128, N], bf16, tag="attn", name="attn")
                nc.vector.tensor_scalar_mul(attn[:], e[:], recip[:])
                nc.sync.dma_start(
                    out=out_ext[t * 128 : (t + 1) * 128, :],
                    in_=attn[:],
                )

    nc.compile()
    return nc


def _get_nc():
    if "nc" not in _cached:
        _cached["nc"] = _build()
    return _cached["nc"]


def _in_maps(fmap1, fmap2, dmap, Wqk):
    bf = ml_dtypes.bfloat16
    f1r = np.asarray(fmap1, dtype=np.float32).reshape(B, C, N)
    f2r = np.asarray(fmap2, dtype=np.float32).reshape(B, C, N)
    dqr = np.asarray(dmap, dtype=np.float32).reshape(B, C, N)
    wT = np.ascontiguousarray(np.asarray(Wqk, dtype=np.float32).T).astype(bf)
    fr = [f1r, f2r]

    in_maps = []
    for i in range(N_CORES):
        b = i // 4
        m = (i % 4) // 2
        h = i % 2
        in_maps.append(
            {
                "fm": np.ascontiguousarray(fr[m][b]).astype(bf),
                "xq": np.ascontiguousarray(
                    dqr[b][:, h * QR : (h + 1) * QR]
                ).astype(bf),
                "wqkT": wT,
            }
        )
    return in_maps


def kernel(fmap1, fmap2, dmap, Wqk):
    from concourse.bass_utils import run_bass_kernel_spmd

    in_maps = _in_maps(fmap1, fmap2, dmap, Wqk)
    nc = _get_nc()
    res = run_bass_kernel_spmd(nc, in_maps, core_ids=list(range(N_CORES)))
    _cached["last_result"] = res

    attns = [
        np.empty((B, 1, N, N), dtype=np.float32),
        np.empty((B, 1, N, N), dtype=np.float32),
    ]
    for i in range(N_CORES):
        b = i // 4
        m = (i % 4) // 2
        h = i % 2
        o = res.results[i]["out"]
        attns[m][b, 0, h * QR : (h + 1) * QR, :] = o.astype(np.float32)
    return (attns[0], attns[1])
